# revision 14
# baseline (speedup 1.0000x reference)
"""Trainium2 Bass kernel for gnn_message_passing (gather + matmul).

Reference computation:
    out[b, m, p] = sum_{c,k} W[m, c*KS+k] * x[b, c, idx[p, k]]
with B=32, C=32, P=4096 pixels, KS=9 neighbors, K=64 output channels.

Strategy (8 NeuronCores, pixel-parallel, direct-HBM gather):
  The host pre-transposes x to xT[p, bc] = x[bc//C, bc%C, p] in bf16, so
  the token for pixel q (all 1024 (b,c) values = 2KB) is a CONTIGUOUS row
  in DRAM.  dma_gather then reads tokens straight from HBM with one
  descriptor per (pixel, k) reference -- no SBUF token table, no
  transpose pass, no 16MB replicated x read (the v1 kernel spent 52us
  building an SBUF table before gathering from it).

  Per core (512 pixels):
   - 18 dma_gather calls (k in 0..9, pixel-half h in {0,1}) of 256 idxs,
     each desc reading xT[idx[p,k], :] (2KB) into
     G[p128, k, h, f, i] = x[bc=f*128+p128, idx[pix, k]]  (bf16, 72KB/par)
     Desc-gen (~2.2us/call on Q7) pipelines with DMA execution; the 16
     physical DMA engines (shared by all queues) are the floor:
     ~9.4MB of 2KB-token reads.
   - Matmuls track the gather per k: block-diagonal weights BD[bp,k]
     (128x128 bf16) map rhs partitions (b', c) -> out partitions (j, m)
     for batches 4f+2bp+j, accumulating k in PSUM (f32), pixel-half h at
     a time (16 accumulators of [128, 2x256] = 8 PSUM banks).
   - PSUM -> SBUF bf16 (DVE cast-copy) -> DRAM out rows (f,bp,j,m),
     cols (h, i); h=0 stores overlap the h=1 gather tail.

  Numbers that shaped this design (HW traces of v1):
   - All SWDGE queues share 16 physical DMA engines (~12GB/s each on
     2KB scattered tokens): gather exec ~= 40us regardless of queue
     count; queue choice only affects desc-gen overlap.
   - SWDGE desc-gen ~= 1us fixed + ~5ns/descriptor per call, serialized
     on the GpSimd queue -> 256-idx calls balance desc-gen (~40us)
     against exec (~43us).
"""

import os

import numpy as np
import ml_dtypes

import concourse.bass as bass
import concourse.mybir as mybir
import concourse.tile as tile
from concourse import bacc
from concourse.bass_utils import run_bass_kernel_spmd

B, C, H, W_IMG = 32, 32, 64, 64
P = H * W_IMG          # 4096 pixels
KS = 9                 # neighbors per pixel
K = 64                 # output channels
NCORES = 8
PPC = P // NCORES      # 512 pixels per core
HPC = PPC // 2         # 256-pixel half (one gather call)
NBC = B * C            # 1024 = full (b, c) dim
NF = NBC // 128        # 8 slabs of 128 (b,c) on the gather free dim
# 4 SWDGE queues overlap gather desc-gen on HW; CoreSim's queue-sem model
# rejects it, so sim validation sets KERNEL_NQUEUES=1.
NQUEUES = int(os.environ.get("KERNEL_NQUEUES", "4"))

_cache = {}


def _build():
    nc = bacc.Bacc("TRN2", target_bir_lowering=False, debug=False,
                   num_devices=NCORES, num_swdge_queues=NQUEUES)

    xT_ext = nc.dram_tensor("xT", [P, NBC], mybir.dt.bfloat16,
                            kind="ExternalInput")
    wbd_ext = nc.dram_tensor("wbd", [128, 2 * KS * 128], mybir.dt.bfloat16,
                             kind="ExternalInput")
    idx_ext = nc.dram_tensor("idx16", [128, KS * PPC // 16], mybir.dt.int16,
                             kind="ExternalInput")
    # Column-major out: [partitions (j,m), (h, fp, bp) x (ff, pix)] so every
    # store writes 2KB contiguous per partition (512B packets otherwise).
    out_ext = nc.dram_tensor("out", [128, B * K * PPC // 128],
                             mybir.dt.bfloat16, kind="ExternalOutput")

    with tile.TileContext(nc) as tc:
        with (
            tc.tile_pool(name="persist", bufs=1) as pp,
            tc.tile_pool(name="stage", bufs=4) as sp,
            tc.tile_pool(name="psmm", bufs=8, space="PSUM") as pmm,
        ):
            idx_t = pp.tile([128, KS * PPC // 16], mybir.dt.int16, tag="idx")
            bd_t = pp.tile([128, 2 * KS, 128], mybir.dt.bfloat16, tag="bd")
            G = pp.tile([128, KS, 2, NF, HPC], mybir.dt.bfloat16, tag="G")
            didx = pp.tile([128, 8], mybir.dt.int16, tag="didx")
            junk = pp.tile([128, NF, 128], mybir.dt.bfloat16, tag="junk")

            # idx gates the first desc-gen -- load it alone, first; the
            # weights are only needed by the matmul phase and load in the
            # shadow of the gather desc-gen.
            nc.sync.dma_start(idx_t[:], idx_ext[:, :])
            nc.sync.dma_start(bd_t[:], wbd_ext[:, :].rearrange(
                "p (a b) -> p a b", b=128))

            # GpSimd busy-chain: the engine otherwise idles ~6->15us before
            # dispatching the first desc-gen (engine init + HAM half-clock);
            # a short memset chain keeps it hot and starts the DVFS ramp
            # while the idx table loads.
            nwarm = int(os.environ.get("KERNEL_NWARM", "10"))
            for i in range(nwarm):
                nc.gpsimd.memset(junk[:, i % NF, :], 0)

            # Gathers: one call per (pixel-half h, neighbor k); descriptor
            # i reads the 2KB row xT[idx[pix_i, k], :].  h-major order so
            # the h=0 matmul phase starts while h=1 is still gathering.
            for h in range(2):
                for k in range(KS):
                    c = 2 * k + h
                    nc.gpsimd.dma_gather(
                        G[:, k, h, :, :],
                        xT_ext[:, :],
                        idx_t[:, c * (HPC // 16):(c + 1) * (HPC // 16)],
                        HPC,        # num_idxs
                        HPC,        # num_idxs_reg (all valid)
                        NBC,        # elem_size (bf16 elements = 2KB row)
                        transpose=True,
                        queue_num=c % NQUEUES,
                    )

            # Matmuls per pixel-half: one accumulator bank per (f-pair,
            # bp) -- the same 128x128 lhsT applies to every f slab, so an
            # f-pair rides the rhs free dim (512-col matmuls, half the
            # instruction count).  4 fpairs x 2 bp = 8 PSUM banks; k-major
            # so the PE consumes each gather as it lands.
            for h in range(2):
                pss = [[pmm.tile([128, 2, HPC], mybir.dt.float32,
                                 name=f"ps{h}_{fp}_{bp}", tag="ps")
                        for bp in range(2)] for fp in range(NF // 2)]
                for k in range(KS):
                    for bp in range(2):
                        for fp in range(NF // 2):
                            nc.tensor.matmul(
                                pss[fp][bp][:],
                                bd_t[:, bp * KS + k, :],
                                G[:, k, h, 2 * fp:2 * fp + 2, :],
                                start=(k == 0),
                                stop=(k == KS - 1),
                            )
                for fp in range(NF // 2):
                    st = sp.tile([128, 2, 2, HPC], mybir.dt.bfloat16,
                                 tag="st")
                    for bp in range(2):
                        nc.vector.tensor_copy(out=st[:, bp],
                                              in_=pss[fp][bp][:])
                    col = (h * 4 + fp) * (2 * 2 * HPC)
                    nc.sync.dma_start(
                        out_ext[:, col:col + 2 * 2 * HPC],
                        st[:].rearrange("p a b c -> p (a b c)"))

    nc.compile()
    return nc


def _get_nc():
    if "nc" not in _cache:
        _cache["nc"] = _build()
    return _cache["nc"]


def _prep_idx16(idx: np.ndarray) -> list:
    """idx (1,64,64,9) int32 -> per-core (128, KS*PPC//16) int16 lists.

    Core i handles pixels [PPC*i, PPC*(i+1)).  Chunk c = 2k+h holds
    idx[p, k] for pixel-half h, wrapped: element j at partition j%16,
    col j//16 (replicated to the 8 16-partition groups)."""
    lst = idx.reshape(P, KS).astype(np.int16)
    outs = []
    for i in range(NCORES):
        o = np.zeros((128, KS * (PPC // 16)), dtype=np.int16)
        for k in range(KS):
            for h in range(2):
                c = 2 * k + h
                lo = PPC * i + h * HPC
                w = lst[lo:lo + HPC, k].reshape(HPC // 16, 16).T
                o[:, c * (HPC // 16):(c + 1) * (HPC // 16)] = \
                    np.tile(w, (8, 1))
        outs.append(o)
    return outs


def _prep_wbd(weights: np.ndarray) -> np.ndarray:
    """weights (64, 288) f32 -> block-diag lhsT set (128, 2*KS*128) bf16.

    BD[bp, k][32*b' + c, 64*j + m] = W[m, c*KS+k] if b' == 2*bp+j else 0,
    for b' in 0..4 (batch-within-group); reused for every group f."""
    bd = np.zeros((2, KS, 128, 128), dtype=np.float32)
    for k in range(KS):
        wk = weights[:, k::KS]  # (64, 32) = W[m, c*KS+k]
        for bp in range(2):
            for j in range(2):
                bprime = 2 * bp + j
                bd[bp, k, 32 * bprime:32 * bprime + 32, 64 * j:64 * j + 64] = \
                    wk.T
    return bd.reshape(2 * KS, 128, 128).transpose(1, 0, 2).reshape(
        128, 2 * KS * 128).astype(ml_dtypes.bfloat16)


def prep_in_maps(x: np.ndarray, weights: np.ndarray, idx: np.ndarray):
    idx16s = _prep_idx16(np.asarray(idx))
    wbd = _prep_wbd(np.asarray(weights, dtype=np.float32))
    # xT[p, bc] = x[bc//C, bc%C, p]: each gather token (all bc for one
    # pixel) is a contiguous 2KB bf16 row in DRAM.
    xT = np.ascontiguousarray(
        np.asarray(x, dtype=np.float32).reshape(NBC, P).T
    ).astype(ml_dtypes.bfloat16)
    return [{"xT": xT, "wbd": wbd, "idx16": idx16s[i]} for i in range(NCORES)]


def assemble_out(results) -> np.ndarray:
    out = np.empty((B, K, P), dtype=np.float32)
    for i in range(NCORES):
        # out_ext[j*64+m, ((h*4+fp)*4 + bp*2 + ff)*HPC + ii]
        r = np.asarray(results[i]["out"]).astype(np.float32).reshape(
            2, K, 2, 4, 2, 2, HPC)  # (j, m, h, fp, bp, ff, ii)
        for fp in range(4):
            for ff in range(2):
                for bp in range(2):
                    for j in range(2):
                        b = 4 * (2 * fp + ff) + 2 * bp + j
                        for h in range(2):
                            lo = PPC * i + h * HPC
                            out[b, :, lo:lo + HPC] = r[j, :, h, fp, bp, ff]
    return out.reshape(B, K, H, W_IMG)


last_results = None


def kernel(x, weights, idx):
    global last_results
    nc = _get_nc()
    in_maps = prep_in_maps(x, weights, idx)
    trace = bool(int(os.environ.get("KERNEL_TRACE", "0")))
    res = run_bass_kernel_spmd(nc, in_maps, core_ids=list(range(NCORES)),
                               trace=trace)
    last_results = res
    return assemble_out(res.results)


# revision 16
# speedup vs baseline: 1.0284x; 1.0284x over previous
"""Trainium2 Bass kernel for gnn_message_passing (gather + matmul).

Reference computation:
    out[b, m, p] = sum_{c,k} W[m, c*KS+k] * x[b, c, idx[p, k]]
with B=32, C=32, P=4096 pixels, KS=9 neighbors, K=64 output channels.

Strategy (8 NeuronCores, pixel-parallel, direct-HBM gather):
  The host pre-transposes x to xT[p, bc] = x[bc//C, bc%C, p] in bf16, so
  the token for pixel q (all 1024 (b,c) values = 2KB) is a CONTIGUOUS row
  in DRAM.  dma_gather then reads tokens straight from HBM with one
  descriptor per (pixel, k) reference -- no SBUF token table, no
  transpose pass, no 16MB replicated x read (the v1 kernel spent 52us
  building an SBUF table before gathering from it).

  Per core (512 pixels):
   - 18 dma_gather calls (k in 0..9, pixel-half h in {0,1}) of 256 idxs,
     each desc reading xT[idx[p,k], :] (2KB) into
     G[p128, k, h, f, i] = x[bc=f*128+p128, idx[pix, k]]  (bf16, 72KB/par)
     Desc-gen (~2.2us/call on Q7) pipelines with DMA execution; the 16
     physical DMA engines (shared by all queues) are the floor:
     ~9.4MB of 2KB-token reads.
   - Matmuls track the gather per k: block-diagonal weights BD[bp,k]
     (128x128 bf16) map rhs partitions (b', c) -> out partitions (j, m)
     for batches 4f+2bp+j, accumulating k in PSUM (f32), pixel-half h at
     a time (16 accumulators of [128, 2x256] = 8 PSUM banks).
   - PSUM -> SBUF bf16 (DVE cast-copy) -> DRAM out rows (f,bp,j,m),
     cols (h, i); h=0 stores overlap the h=1 gather tail.

  Numbers that shaped this design (HW traces of v1):
   - All SWDGE queues share 16 physical DMA engines (~12GB/s each on
     2KB scattered tokens): gather exec ~= 40us regardless of queue
     count; queue choice only affects desc-gen overlap.
   - SWDGE desc-gen ~= 1us fixed + ~5ns/descriptor per call, serialized
     on the GpSimd queue -> 256-idx calls balance desc-gen (~40us)
     against exec (~43us).
"""

import os

import numpy as np
import ml_dtypes

import concourse.bass as bass
import concourse.mybir as mybir
import concourse.tile as tile
from concourse import bacc
from concourse.bass_utils import run_bass_kernel_spmd

B, C, H, W_IMG = 32, 32, 64, 64
P = H * W_IMG          # 4096 pixels
KS = 9                 # neighbors per pixel
K = 64                 # output channels
NCORES = 8
PPC = P // NCORES      # 512 pixels per core
HPC = PPC // 2         # 256-pixel half (one gather call)
NBC = B * C            # 1024 = full (b, c) dim
NF = NBC // 128        # 8 slabs of 128 (b,c) on the gather free dim
# 4 SWDGE queues overlap gather desc-gen on HW; CoreSim's queue-sem model
# rejects it, so sim validation sets KERNEL_NQUEUES=1.
NQUEUES = int(os.environ.get("KERNEL_NQUEUES", "4"))

_cache = {}


def _build():
    nc = bacc.Bacc("TRN2", target_bir_lowering=False, debug=False,
                   num_devices=NCORES, num_swdge_queues=NQUEUES)

    xT_ext = nc.dram_tensor("xT", [P, NBC], mybir.dt.bfloat16,
                            kind="ExternalInput")
    wbd_ext = nc.dram_tensor("wbd", [128, 2 * KS * 128], mybir.dt.bfloat16,
                             kind="ExternalInput")
    idx_ext = nc.dram_tensor("idx16", [128, KS * PPC // 16], mybir.dt.int16,
                             kind="ExternalInput")
    # Column-major out: [partitions (j,m), (h, fp, bp) x (ff, pix)] so every
    # store writes 2KB contiguous per partition (512B packets otherwise).
    out_ext = nc.dram_tensor("out", [128, B * K * PPC // 128],
                             mybir.dt.bfloat16, kind="ExternalOutput")

    with tile.TileContext(nc) as tc:
        with (
            tc.tile_pool(name="persist", bufs=1) as pp,
            tc.tile_pool(name="stage", bufs=4) as sp,
            tc.tile_pool(name="psmm", bufs=8, space="PSUM") as pmm,
        ):
            idx_t = pp.tile([128, KS * PPC // 16], mybir.dt.int16, tag="idx")
            bd_t = pp.tile([128, 2 * KS, 128], mybir.dt.bfloat16, tag="bd")
            G = pp.tile([128, KS, 2, NF, HPC], mybir.dt.bfloat16, tag="G")
            didx = pp.tile([128, 8], mybir.dt.int16, tag="didx")
            junk = pp.tile([128, NF, 128], mybir.dt.bfloat16, tag="junk")

            # idx gates the first desc-gen -- load it alone, first; the
            # weights are only needed by the matmul phase and load in the
            # shadow of the gather desc-gen.
            nc.sync.dma_start(idx_t[:], idx_ext[:, :])
            nc.sync.dma_start(bd_t[:], wbd_ext[:, :].rearrange(
                "p (a b) -> p a b", b=128))

            # Q7 boot op: the first dma_gather custom op pays ~11us of
            # GPSIMD DSP boot before its desc-gen runs.  A tiny
            # SBUF-to-SBUF gather (reads/writes only scratch, queue 3)
            # absorbs that cost while the idx table is still loading.
            junk2 = pp.tile([128, 1, 128], mybir.dt.bfloat16, tag="junk2")
            nc.gpsimd.memset(didx[:], 0)
            nc.gpsimd.memset(junk[:], 0)
            if int(os.environ.get("KERNEL_BOOT", "1")):
                nc.gpsimd.dma_gather(
                    junk2[:],
                    junk[:].rearrange("p a b -> p (a b)"),
                    didx[:, :8], 128, 128, 128,
                    transpose=True,
                    sbuf_tokens_per_rank=128,
                    sbuf_free_dim_per_rank=2048,
                    queue_num=min(3, NQUEUES - 1),
                )

            # Gathers: one call per (pixel-half h, neighbor k); descriptor
            # i reads the 2KB row xT[idx[pix_i, k], :].  h-major order so
            # the h=0 matmul phase starts while h=1 is still gathering.
            for h in range(2):
                for k in range(KS):
                    c = 2 * k + h
                    nc.gpsimd.dma_gather(
                        G[:, k, h, :, :],
                        xT_ext[:, :],
                        idx_t[:, c * (HPC // 16):(c + 1) * (HPC // 16)],
                        HPC,        # num_idxs
                        HPC,        # num_idxs_reg (all valid)
                        NBC,        # elem_size (bf16 elements = 2KB row)
                        transpose=True,
                        queue_num=c % NQUEUES,
                    )

            # Matmuls per pixel-half: one accumulator bank per (f-pair,
            # bp) -- the same 128x128 lhsT applies to every f slab, so an
            # f-pair rides the rhs free dim (512-col matmuls, half the
            # instruction count).  4 fpairs x 2 bp = 8 PSUM banks; k-major
            # so the PE consumes each gather as it lands.
            for h in range(2):
                pss = [[pmm.tile([128, 2, HPC], mybir.dt.float32,
                                 name=f"ps{h}_{fp}_{bp}", tag="ps")
                        for bp in range(2)] for fp in range(NF // 2)]
                for k in range(KS):
                    for bp in range(2):
                        for fp in range(NF // 2):
                            nc.tensor.matmul(
                                pss[fp][bp][:],
                                bd_t[:, bp * KS + k, :],
                                G[:, k, h, 2 * fp:2 * fp + 2, :],
                                start=(k == 0),
                                stop=(k == KS - 1),
                            )
                for fp in range(NF // 2):
                    st = sp.tile([128, 2, 2, HPC], mybir.dt.bfloat16,
                                 tag="st")
                    for bp in range(2):
                        nc.vector.tensor_copy(out=st[:, bp],
                                              in_=pss[fp][bp][:])
                    col = (h * 4 + fp) * (2 * 2 * HPC)
                    nc.sync.dma_start(
                        out_ext[:, col:col + 2 * 2 * HPC],
                        st[:].rearrange("p a b c -> p (a b c)"))

    nc.compile()
    return nc


def _get_nc():
    if "nc" not in _cache:
        _cache["nc"] = _build()
    return _cache["nc"]


def _prep_idx16(idx: np.ndarray) -> list:
    """idx (1,64,64,9) int32 -> per-core (128, KS*PPC//16) int16 lists.

    Core i handles pixels [PPC*i, PPC*(i+1)).  Chunk c = 2k+h holds
    idx[p, k] for pixel-half h, wrapped: element j at partition j%16,
    col j//16 (replicated to the 8 16-partition groups)."""
    lst = idx.reshape(P, KS).astype(np.int16)
    outs = []
    for i in range(NCORES):
        o = np.zeros((128, KS * (PPC // 16)), dtype=np.int16)
        for k in range(KS):
            for h in range(2):
                c = 2 * k + h
                lo = PPC * i + h * HPC
                w = lst[lo:lo + HPC, k].reshape(HPC // 16, 16).T
                o[:, c * (HPC // 16):(c + 1) * (HPC // 16)] = \
                    np.tile(w, (8, 1))
        outs.append(o)
    return outs


def _prep_wbd(weights: np.ndarray) -> np.ndarray:
    """weights (64, 288) f32 -> block-diag lhsT set (128, 2*KS*128) bf16.

    BD[bp, k][32*b' + c, 64*j + m] = W[m, c*KS+k] if b' == 2*bp+j else 0,
    for b' in 0..4 (batch-within-group); reused for every group f."""
    bd = np.zeros((2, KS, 128, 128), dtype=np.float32)
    for k in range(KS):
        wk = weights[:, k::KS]  # (64, 32) = W[m, c*KS+k]
        for bp in range(2):
            for j in range(2):
                bprime = 2 * bp + j
                bd[bp, k, 32 * bprime:32 * bprime + 32, 64 * j:64 * j + 64] = \
                    wk.T
    return bd.reshape(2 * KS, 128, 128).transpose(1, 0, 2).reshape(
        128, 2 * KS * 128).astype(ml_dtypes.bfloat16)


def prep_in_maps(x: np.ndarray, weights: np.ndarray, idx: np.ndarray):
    idx16s = _prep_idx16(np.asarray(idx))
    wbd = _prep_wbd(np.asarray(weights, dtype=np.float32))
    # xT[p, bc] = x[bc//C, bc%C, p]: each gather token (all bc for one
    # pixel) is a contiguous 2KB bf16 row in DRAM.
    xT = np.ascontiguousarray(
        np.asarray(x, dtype=np.float32).reshape(NBC, P).T
    ).astype(ml_dtypes.bfloat16)
    return [{"xT": xT, "wbd": wbd, "idx16": idx16s[i]} for i in range(NCORES)]


def assemble_out(results) -> np.ndarray:
    out = np.empty((B, K, P), dtype=np.float32)
    for i in range(NCORES):
        # out_ext[j*64+m, ((h*4+fp)*4 + bp*2 + ff)*HPC + ii]
        r = np.asarray(results[i]["out"]).astype(np.float32).reshape(
            2, K, 2, 4, 2, 2, HPC)  # (j, m, h, fp, bp, ff, ii)
        for fp in range(4):
            for ff in range(2):
                for bp in range(2):
                    for j in range(2):
                        b = 4 * (2 * fp + ff) + 2 * bp + j
                        for h in range(2):
                            lo = PPC * i + h * HPC
                            out[b, :, lo:lo + HPC] = r[j, :, h, fp, bp, ff]
    return out.reshape(B, K, H, W_IMG)


last_results = None


def kernel(x, weights, idx):
    global last_results
    nc = _get_nc()
    in_maps = prep_in_maps(x, weights, idx)
    trace = bool(int(os.environ.get("KERNEL_TRACE", "0")))
    res = run_bass_kernel_spmd(nc, in_maps, core_ids=list(range(NCORES)),
                               trace=trace)
    last_results = res
    return assemble_out(res.results)


# revision 19
# speedup vs baseline: 1.0502x; 1.0212x over previous
"""Trainium2 Bass kernel for gnn_message_passing (gather + matmul).

Reference computation:
    out[b, m, p] = sum_{c,k} W[m, c*KS+k] * x[b, c, idx[p, k]]
with B=32, C=32, P=4096 pixels, KS=9 neighbors, K=64 output channels.

Strategy (8 NeuronCores, pixel-parallel, direct-HBM gather):
  The host pre-transposes x to xT[p, bc] = x[bc//C, bc%C, p] in bf16, so
  the token for pixel q (all 1024 (b,c) values = 2KB) is a CONTIGUOUS row
  in DRAM.  dma_gather then reads tokens straight from HBM with one
  descriptor per (pixel, k) reference -- no SBUF token table, no
  transpose pass, no 16MB replicated x read (the v1 kernel spent 52us
  building an SBUF table before gathering from it).

  Per core (512 pixels):
   - 18 dma_gather calls (k in 0..9, pixel-half h in {0,1}) of 256 idxs,
     each desc reading xT[idx[p,k], :] (2KB) into
     G[p128, k, h, f, i] = x[bc=f*128+p128, idx[pix, k]]  (bf16, 72KB/par)
     Desc-gen (~2.2us/call on Q7) pipelines with DMA execution; the 16
     physical DMA engines (shared by all queues) are the floor:
     ~9.4MB of 2KB-token reads.
   - Matmuls track the gather per k: block-diagonal weights BD[bp,k]
     (128x128 bf16) map rhs partitions (b', c) -> out partitions (j, m)
     for batches 4f+2bp+j, accumulating k in PSUM (f32), pixel-half h at
     a time (16 accumulators of [128, 2x256] = 8 PSUM banks).
   - PSUM -> SBUF bf16 (DVE cast-copy) -> DRAM out rows (f,bp,j,m),
     cols (h, i); h=0 stores overlap the h=1 gather tail.

  Numbers that shaped this design (HW traces of v1):
   - All SWDGE queues share 16 physical DMA engines (~12GB/s each on
     2KB scattered tokens): gather exec ~= 40us regardless of queue
     count; queue choice only affects desc-gen overlap.
   - SWDGE desc-gen ~= 1us fixed + ~5ns/descriptor per call, serialized
     on the GpSimd queue -> 256-idx calls balance desc-gen (~40us)
     against exec (~43us).
"""

import os

import numpy as np
import ml_dtypes

import concourse.bass as bass
import concourse.mybir as mybir
import concourse.tile as tile
from concourse import bacc
from concourse.bass_utils import run_bass_kernel_spmd

B, C, H, W_IMG = 32, 32, 64, 64
P = H * W_IMG          # 4096 pixels
KS = 9                 # neighbors per pixel
K = 64                 # output channels
NCORES = 8
PPC = P // NCORES      # 512 pixels per core
HPC = PPC // 2         # 256-pixel half (one gather call)
NBC = B * C            # 1024 = full (b, c) dim
NF = NBC // 128        # 8 slabs of 128 (b,c) on the gather free dim
# 4 SWDGE queues overlap gather desc-gen on HW; CoreSim's queue-sem model
# rejects it, so sim validation sets KERNEL_NQUEUES=1.
NQUEUES = int(os.environ.get("KERNEL_NQUEUES", "4"))

_cache = {}


def _build():
    nc = bacc.Bacc("TRN2", target_bir_lowering=False, debug=False,
                   num_devices=NCORES, num_swdge_queues=NQUEUES)

    xT_ext = nc.dram_tensor("xT", [P, NBC], mybir.dt.bfloat16,
                            kind="ExternalInput")
    wbd_ext = nc.dram_tensor("wbd", [128, 2 * KS * 128], mybir.dt.bfloat16,
                             kind="ExternalInput")
    idx_ext = nc.dram_tensor("idx16", [128, KS * PPC // 16], mybir.dt.int16,
                             kind="ExternalInput")
    # Column-major out: [partitions (j,m), (h, fp, bp) x (ff, pix)] so every
    # store writes 2KB contiguous per partition (512B packets otherwise).
    out_ext = nc.dram_tensor("out", [128, B * K * PPC // 128],
                             mybir.dt.bfloat16, kind="ExternalOutput")

    with tile.TileContext(nc) as tc:
        with (
            tc.tile_pool(name="persist", bufs=1) as pp,
            tc.tile_pool(name="stage", bufs=4) as sp,
            tc.tile_pool(name="psmm", bufs=8, space="PSUM") as pmm,
        ):
            idx_t = pp.tile([128, KS * PPC // 16], mybir.dt.int16, tag="idx")
            bd_t = pp.tile([128, 2 * KS, 128], mybir.dt.bfloat16, tag="bd")
            G = pp.tile([128, KS, 2, NF, HPC], mybir.dt.bfloat16, tag="G")

            # idx gates the first desc-gen -- load it alone, first; the
            # weights are only needed by the matmul phase and load in the
            # shadow of the gather desc-gen.  (The first desc-gen still
            # can't run before ~18us: the GPSIMD Q7 DSPs boot
            # asynchronously from kernel start and no custom op -- even a
            # dependency-free one -- dispatches earlier; measured, not
            # fixable from the instruction stream.)
            nc.sync.dma_start(idx_t[:], idx_ext[:, :])
            nc.sync.dma_start(bd_t[:], wbd_ext[:, :].rearrange(
                "p (a b) -> p a b", b=128))

            # Gathers: one call per (pixel-half h, neighbor k); descriptor
            # i reads the 2KB row xT[idx[pix_i, k], :].  h-major order so
            # the h=0 matmul phase starts while h=1 is still gathering.
            for h in range(2):
                for k in range(KS):
                    c = 2 * k + h
                    nc.gpsimd.dma_gather(
                        G[:, k, h, :, :],
                        xT_ext[:, :],
                        idx_t[:, c * (HPC // 16):(c + 1) * (HPC // 16)],
                        HPC,        # num_idxs
                        HPC,        # num_idxs_reg (all valid)
                        NBC,        # elem_size (bf16 elements = 2KB row)
                        transpose=True,
                        queue_num=c % NQUEUES,
                    )

            # Matmuls per pixel-half: one accumulator bank per (f-pair,
            # bp) -- the same 128x128 lhsT applies to every f slab, so an
            # f-pair rides the rhs free dim (512-col matmuls, half the
            # instruction count).  4 fpairs x 2 bp = 8 PSUM banks; k-major
            # so the PE consumes each gather as it lands.
            for h in range(2):
                pss = [[pmm.tile([128, 2, HPC], mybir.dt.float32,
                                 name=f"ps{h}_{fp}_{bp}", tag="ps")
                        for bp in range(2)] for fp in range(NF // 2)]
                for k in range(KS):
                    for bp in range(2):
                        for fp in range(NF // 2):
                            nc.tensor.matmul(
                                pss[fp][bp][:],
                                bd_t[:, bp * KS + k, :],
                                G[:, k, h, 2 * fp:2 * fp + 2, :],
                                start=(k == 0),
                                stop=(k == KS - 1),
                            )
                for fp in range(NF // 2):
                    st = sp.tile([128, 2, 2, HPC], mybir.dt.bfloat16,
                                 tag="st")
                    for bp in range(2):
                        nc.vector.tensor_copy(out=st[:, bp],
                                              in_=pss[fp][bp][:])
                    col = (h * 4 + fp) * (2 * 2 * HPC)
                    nc.sync.dma_start(
                        out_ext[:, col:col + 2 * 2 * HPC],
                        st[:].rearrange("p a b c -> p (a b c)"))

    nc.compile()
    return nc


def _get_nc():
    if "nc" not in _cache:
        _cache["nc"] = _build()
    return _cache["nc"]


def _prep_idx16(idx: np.ndarray) -> list:
    """idx (1,64,64,9) int32 -> per-core (128, KS*PPC//16) int16 lists.

    Core i handles pixels [PPC*i, PPC*(i+1)).  Chunk c = 2k+h holds
    idx[p, k] for pixel-half h, wrapped: element j at partition j%16,
    col j//16 (replicated to the 8 16-partition groups)."""
    lst = idx.reshape(P, KS).astype(np.int16)
    outs = []
    for i in range(NCORES):
        o = np.zeros((128, KS * (PPC // 16)), dtype=np.int16)
        for k in range(KS):
            for h in range(2):
                c = 2 * k + h
                lo = PPC * i + h * HPC
                w = lst[lo:lo + HPC, k].reshape(HPC // 16, 16).T
                o[:, c * (HPC // 16):(c + 1) * (HPC // 16)] = \
                    np.tile(w, (8, 1))
        outs.append(o)
    return outs


def _prep_wbd(weights: np.ndarray) -> np.ndarray:
    """weights (64, 288) f32 -> block-diag lhsT set (128, 2*KS*128) bf16.

    BD[bp, k][32*b' + c, 64*j + m] = W[m, c*KS+k] if b' == 2*bp+j else 0,
    for b' in 0..4 (batch-within-group); reused for every group f."""
    bd = np.zeros((2, KS, 128, 128), dtype=np.float32)
    for k in range(KS):
        wk = weights[:, k::KS]  # (64, 32) = W[m, c*KS+k]
        for bp in range(2):
            for j in range(2):
                bprime = 2 * bp + j
                bd[bp, k, 32 * bprime:32 * bprime + 32, 64 * j:64 * j + 64] = \
                    wk.T
    return bd.reshape(2 * KS, 128, 128).transpose(1, 0, 2).reshape(
        128, 2 * KS * 128).astype(ml_dtypes.bfloat16)


def prep_in_maps(x: np.ndarray, weights: np.ndarray, idx: np.ndarray):
    idx16s = _prep_idx16(np.asarray(idx))
    wbd = _prep_wbd(np.asarray(weights, dtype=np.float32))
    # xT[p, bc] = x[bc//C, bc%C, p]: each gather token (all bc for one
    # pixel) is a contiguous 2KB bf16 row in DRAM.
    xT = np.ascontiguousarray(
        np.asarray(x, dtype=np.float32).reshape(NBC, P).T
    ).astype(ml_dtypes.bfloat16)
    return [{"xT": xT, "wbd": wbd, "idx16": idx16s[i]} for i in range(NCORES)]


def assemble_out(results) -> np.ndarray:
    out = np.empty((B, K, P), dtype=np.float32)
    for i in range(NCORES):
        # out_ext[j*64+m, ((h*4+fp)*4 + bp*2 + ff)*HPC + ii]
        r = np.asarray(results[i]["out"]).astype(np.float32).reshape(
            2, K, 2, 4, 2, 2, HPC)  # (j, m, h, fp, bp, ff, ii)
        for fp in range(4):
            for ff in range(2):
                for bp in range(2):
                    for j in range(2):
                        b = 4 * (2 * fp + ff) + 2 * bp + j
                        for h in range(2):
                            lo = PPC * i + h * HPC
                            out[b, :, lo:lo + HPC] = r[j, :, h, fp, bp, ff]
    return out.reshape(B, K, H, W_IMG)


last_results = None


def kernel(x, weights, idx):
    global last_results
    nc = _get_nc()
    in_maps = prep_in_maps(x, weights, idx)
    trace = bool(int(os.environ.get("KERNEL_TRACE", "0")))
    res = run_bass_kernel_spmd(nc, in_maps, core_ids=list(range(NCORES)),
                               trace=trace)
    last_results = res
    return assemble_out(res.results)
